# revision 1
# baseline (speedup 1.0000x reference)
"""Trainium2 Bass kernel for CustomGINE message passing (8 NeuronCores).

Strategy:
  - Nodes are sharded by destination across the 8 cores (12500 nodes each);
    each core receives exactly the edges whose dst falls in its node range,
    so the per-node aggregation is fully local (no big all-reduce).
  - Within a core, edge slots are laid out by (src supergroup q of 32768
    nodes, dst block b of 128 nodes) so that:
      * x[src] rows (256B each) are fetched with big `dma_gather` calls
        using int16 indices relative to the supergroup base, and
      * the segment-sum becomes, per 128-edge tile, one matmul
        msg^T @ S where S[e, j] = (dst_e == block_start + j) is built
        on-device in bf16 with a tensor-tensor is_equal.
  - edge_attr embedding rows are added via a K=4 one-hot matmul into the
    same PSUM accumulation (hi+lo bf16 split of the embedding table keeps
    full precision).
  - The MLP tail runs in feature-major (transposed) layout so BatchNorm
    statistics are free-dim reductions; the cross-core BN mean/var
    reduction is a tiny [64, 2] AllReduce collective.

Only index/layout work happens on the host (sorting, padding, one-hot
encodings, dtype of index tensors); all floating-point math runs on device.
"""

import os
import sys

for _p in ("/opt/trn_rl_repo", "/root/.axon_site/_ro/trn_rl_repo"):
    if os.path.isdir(_p) and _p not in sys.path:
        sys.path.insert(0, _p)

import numpy as np
import ml_dtypes

BF16 = ml_dtypes.bfloat16

N_NODES = 100000
IN_DIM = 64
NCORES = 8
NPC = N_NODES // NCORES          # nodes per core
P = 128                          # partitions
NBLK = (NPC + P - 1) // P        # dst blocks of 128 nodes per core (98)
NLOC = NBLK * P                  # padded local node count (12544)
Q = 4                            # src supergroups
QS = 25000                       # supergroup stride (balanced; < 32768)
QW = 32768                       # gather window rows per supergroup
XROWS = 3 * QS + QW              # padded gather table rows (107768)
BN_EPS = 1e-5

_PROGRAM_CACHE = {}


def _host_prepare(x, edge_index, edge_attr):
    """Shard + lay out edges; returns per-core index/metadata arrays."""
    src = np.asarray(edge_index[0], dtype=np.int64)
    dst = np.asarray(edge_index[1], dtype=np.int64)
    attr = np.asarray(edge_attr, dtype=np.int64)
    E = src.shape[0]

    core = dst // NPC
    q = src // QS
    dloc = dst - core * NPC
    b = dloc >> 7

    # order edges by (core, q, b); within a cell order is irrelevant
    key = (core * Q + q) * NBLK + b
    order = np.argsort(key, kind="stable")
    src_s, dloc_s, attr_s, key_s = src[order], dloc[order], attr[order], key[order]

    # per-(core, q, b) counts -> uniform tile capacity per cell
    counts = np.bincount(key_s, minlength=NCORES * Q * NBLK)
    tqb = int((counts.max() + P - 1) // P)
    tqb = max(tqb, 1)

    tiles_per_q = NBLK * tqb
    T = Q * tiles_per_q                  # tiles per core
    SLOTS = T * P                        # edge slots per core
    cell_cap = tqb * P

    # slot id within a core: cell (q, b) occupies [ (q*NBLK+b)*cell_cap, ... )
    cell_ix = np.zeros(NCORES * Q * NBLK + 1, dtype=np.int64)
    cell_ix[1:] = np.cumsum(counts)
    # position of each (sorted) edge within its cell
    pos_in_cell = np.arange(E, dtype=np.int64) - cell_ix[key_s]
    cell_of_edge = key_s
    core_of_edge = cell_of_edge // (Q * NBLK)
    celllocal = cell_of_edge % (Q * NBLK)
    slot = celllocal * cell_cap + pos_in_cell   # slot within the core

    idx16 = np.zeros((NCORES, SLOTS), dtype=np.int16)
    dstrel = np.full((NCORES, SLOTS), -1.0, dtype=BF16)
    attr1h = np.zeros((NCORES, 4, SLOTS), dtype=BF16)

    c_arr = core_of_edge
    idx16[c_arr, slot] = (src_s - q[order] * QS).astype(np.int16)
    dstrel[c_arr, slot] = (dloc_s & 127).astype(np.float32).astype(BF16)
    attr1h[c_arr, attr_s, slot] = BF16(1.0)

    # wrap idx16 per gather call: slot i (within call) -> [i % 16, i // 16]
    calls_q = 14 if (tiles_per_q % 14 == 0) else 7
    while tiles_per_q % calls_q != 0:
        calls_q -= 1
    call_tiles = tiles_per_q // calls_q
    call_slots = call_tiles * P
    if call_slots > 8192:
        # split calls further if the per-call index count gets too large
        for cq in range(calls_q + 1, tiles_per_q + 1):
            if tiles_per_q % cq == 0 and (tiles_per_q // cq) * P <= 8192:
                calls_q = cq
                call_tiles = tiles_per_q // cq
                call_slots = call_tiles * P
                break
    ncalls = Q * calls_q

    idx_w = idx16.reshape(NCORES, ncalls, call_slots // 16, 16)
    idx_w = np.ascontiguousarray(np.swapaxes(idx_w, 2, 3))  # [NC, ncalls, 16, cs/16]
    idx_wrapped = idx_w.reshape(NCORES, ncalls * 16, call_slots // 16)
    # final DRAM layout: [128, SLOTS/16] where call c occupies columns
    # [c*cs/16, (c+1)*cs/16) and its 16 rows are replicated 8x down partitions
    idx_dram = np.zeros((NCORES, P, SLOTS // 16), dtype=np.int16)
    for c in range(ncalls):
        blkc = idx_wrapped[:, c * 16:(c + 1) * 16, :]           # [NC, 16, cs/16]
        cols = slice(c * (call_slots // 16), (c + 1) * (call_slots // 16))
        idx_dram[:, :, cols] = np.tile(blkc, (1, 8, 1))

    dstrel_mat = np.ascontiguousarray(
        dstrel.reshape(NCORES, T, P).swapaxes(1, 2))       # [NC, 128, T]

    meta = dict(tqb=tqb, T=T, SLOTS=SLOTS, calls_q=calls_q,
                call_tiles=call_tiles, call_slots=call_slots)
    return idx_dram, dstrel_mat, attr1h, meta


def _build_program(meta, emb_lo_split=True):
    import concourse.bass as bass
    import concourse.bacc as bacc
    import concourse.mybir as mybir
    import concourse.tile as tile

    dt = mybir.dt
    Alu = mybir.AluOpType
    Act = mybir.ActivationFunctionType

    tqb = meta["tqb"]
    T = meta["T"]
    SLOTS = meta["SLOTS"]
    calls_q = meta["calls_q"]
    call_tiles = meta["call_tiles"]
    call_slots = meta["call_slots"]
    CHUNK = 7 if call_tiles % 7 == 0 else 1
    while call_tiles % CHUNK != 0:
        CHUNK -= 1
    chunks_per_call = call_tiles // CHUNK

    nc = bacc.Bacc("TRN2", target_bir_lowering=False, debug=False,
                   num_devices=NCORES)
    _aggps = {}

    f32, bf16, i16 = dt.float32, dt.bfloat16, dt.int16

    xg = nc.dram_tensor("xg", [XROWS, IN_DIM], f32, kind="ExternalInput")
    xloc = nc.dram_tensor("xloc", [NLOC, IN_DIM], f32, kind="ExternalInput")
    idx16 = nc.dram_tensor("idx16", [P, SLOTS // 16], i16, kind="ExternalInput")
    dstrel = nc.dram_tensor("dstrel", [P, T], bf16, kind="ExternalInput")
    attr1h = nc.dram_tensor("attr1h", [4, SLOTS], bf16, kind="ExternalInput")
    iota_d = nc.dram_tensor("iota", [P, P], bf16, kind="ExternalInput")
    ident_d = nc.dram_tensor("ident", [P, P], f32, kind="ExternalInput")
    emb_d = nc.dram_tensor("emb", [4, IN_DIM], f32, kind="ExternalInput")
    w1_d = nc.dram_tensor("w1", [IN_DIM, IN_DIM], f32, kind="ExternalInput")
    w2_d = nc.dram_tensor("w2", [IN_DIM, IN_DIM], f32, kind="ExternalInput")
    gam_d = nc.dram_tensor("gam", [IN_DIM, 1], f32, kind="ExternalInput")
    bet_d = nc.dram_tensor("bet", [IN_DIM, 1], f32, kind="ExternalInput")
    b2_d = nc.dram_tensor("b2", [IN_DIM, 1], f32, kind="ExternalInput")
    eps_d = nc.dram_tensor("eps", [P, 1], f32, kind="ExternalInput")
    out_d = nc.dram_tensor("outT", [IN_DIM, NLOC], f32, kind="ExternalOutput")

    cc_in = nc.dram_tensor("cc_in", [IN_DIM, 2], f32)
    cc_out = nc.dram_tensor("cc_out", [IN_DIM, 2], f32, addr_space="Shared")

    with tile.TileContext(nc) as tc:
        with (
            tc.tile_pool(name="const", bufs=1) as cpool,
            tc.tile_pool(name="big", bufs=1) as bigpool,
            tc.tile_pool(name="gin", bufs=2) as gpool,
            tc.tile_pool(name="meta", bufs=2) as mpool,
            tc.tile_pool(name="work", bufs=3) as wpool,
            tc.tile_pool(name="psum", bufs=2, space="PSUM") as pp,
            tc.tile_pool(name="psagg", bufs=4, space="PSUM") as ppagg,
        ):
            # ---- constants / params ----
            iota_t = cpool.tile([P, P], bf16)
            nc.sync.dma_start(out=iota_t[:], in_=iota_d[:])
            ident_t = cpool.tile([P, P], f32)
            nc.sync.dma_start(out=ident_t[:], in_=ident_d[:])
            dstrel_t = cpool.tile([P, T], bf16)
            nc.sync.dma_start(out=dstrel_t[:], in_=dstrel[:])
            emb_t = cpool.tile([4, IN_DIM], f32)
            nc.sync.dma_start(out=emb_t[:], in_=emb_d[:])
            w1_t = cpool.tile([IN_DIM, IN_DIM], f32)
            nc.sync.dma_start(out=w1_t[:], in_=w1_d[:])
            w2_t = cpool.tile([IN_DIM, IN_DIM], f32)
            nc.sync.dma_start(out=w2_t[:], in_=w2_d[:])
            gam_t = cpool.tile([IN_DIM, 1], f32)
            nc.sync.dma_start(out=gam_t[:], in_=gam_d[:])
            bet_t = cpool.tile([IN_DIM, 1], f32)
            nc.sync.dma_start(out=bet_t[:], in_=bet_d[:])
            b2_t = cpool.tile([IN_DIM, 1], f32)
            nc.sync.dma_start(out=b2_t[:], in_=b2_d[:])
            eps_t = cpool.tile([P, 1], f32)
            nc.sync.dma_start(out=eps_t[:], in_=eps_d[:])

            # emb hi/lo bf16 split (exact to ~2^-17)
            emb_hi = cpool.tile([4, IN_DIM], bf16)
            nc.vector.tensor_copy(out=emb_hi[:], in_=emb_t[:])
            emb_lo = cpool.tile([4, IN_DIM], bf16)
            if emb_lo_split:
                emb_hi_f = cpool.tile([4, IN_DIM], f32)
                nc.vector.tensor_copy(out=emb_hi_f[:], in_=emb_hi[:])
                emb_lo_f = cpool.tile([4, IN_DIM], f32)
                nc.vector.tensor_tensor(out=emb_lo_f[:], in0=emb_t[:],
                                        in1=emb_hi_f[:], op=Alu.subtract)
                nc.vector.tensor_copy(out=emb_lo[:], in_=emb_lo_f[:])

            # Ieps = (1 + eps) * I  (f32, exact for any eps)
            eps1_t = cpool.tile([P, 1], f32)
            nc.scalar.activation(out=eps1_t[:], in_=eps_t[:], func=Act.Identity,
                                 bias=1.0, scale=1.0)
            ieps_t = cpool.tile([P, P], f32)
            nc.vector.tensor_scalar(ieps_t[:], ident_t[:], eps1_t[:, :1], None,
                                    Alu.mult)

            # aggregated h^T accumulator [64, NLOC]
            agg_sb = bigpool.tile([IN_DIM, NLOC], f32)
            nc.vector.memset(agg_sb[:], 0.0)

            # ---- edge phase ----
            for qi in range(Q):
                for ci in range(calls_q):
                    call = qi * calls_q + ci
                    t0 = call * call_tiles          # first tile of this call
                    s0 = t0 * P                     # first slot
                    idx_sb = mpool.tile([P, call_slots // 16], i16, tag="idx")
                    nc.sync.dma_start(
                        out=idx_sb[:],
                        in_=idx16[:, call * (call_slots // 16):
                                  (call + 1) * (call_slots // 16)])
                    gbuf = gpool.tile([P, call_tiles * IN_DIM], f32, tag="g")
                    nc.gpsimd.dma_gather(
                        out_ap=gbuf[:].rearrange("p (k f) -> p k f", f=IN_DIM),
                        in_ap=xg[qi * QS:qi * QS + QW, :],
                        idxs_ap=idx_sb[:],
                        num_idxs=call_slots,
                        num_idxs_reg=call_slots,
                        elem_size=IN_DIM,
                        single_packet=False,
                    )
                    for ch in range(chunks_per_call):
                        # tiles [t0 + ch*CHUNK, t0 + (ch+1)*CHUNK)
                        tb = t0 + ch * CHUNK
                        at_sb = mpool.tile([4, CHUNK * P], bf16, tag="attr")
                        nc.sync.dma_start(
                            out=at_sb[:],
                            in_=attr1h[:, tb * P:(tb + CHUNK) * P])
                        ps_pre = pp.tile([P, CHUNK * IN_DIM], f32, space="PSUM",
                                         tag="pre")
                        for j in range(CHUNK):
                            acol = j * P
                            lhs = at_sb[:, acol:acol + P]
                            nc.tensor.matmul(
                                out=ps_pre[:, j * IN_DIM:(j + 1) * IN_DIM],
                                lhsT=lhs, rhs=emb_hi[:],
                                start=True, stop=not emb_lo_split)
                            if emb_lo_split:
                                nc.tensor.matmul(
                                    out=ps_pre[:, j * IN_DIM:(j + 1) * IN_DIM],
                                    lhsT=lhs, rhs=emb_lo[:],
                                    start=False, stop=True)
                        # msg = relu(gx + emb_one_hot @ emb)
                        gsl = gbuf[:, ch * CHUNK * IN_DIM:
                                   (ch + 1) * CHUNK * IN_DIM]
                        madd = wpool.tile([P, CHUNK * IN_DIM], f32, tag="madd")
                        nc.vector.tensor_tensor(out=madd[:], in0=gsl,
                                                in1=ps_pre[:], op=Alu.add)
                        msg = wpool.tile([P, CHUNK * IN_DIM], bf16, tag="msg")
                        nc.scalar.activation(out=msg[:], in_=madd[:],
                                             func=Act.Relu)
                        # S[e, j] = (dstrel_e == j), bf16 0/1
                        s_t = wpool.tile([P, CHUNK * P], bf16, tag="S")
                        io_b = iota_t[:].rearrange("p (o f) -> p o f", o=1).to_broadcast(
                            [P, CHUNK, P])
                        dr_b = dstrel_t[:, tb:tb + CHUNK].rearrange(
                            "p (k o) -> p k o", o=1).to_broadcast([P, CHUNK, P])
                        nc.vector.tensor_tensor(
                            out=s_t[:].rearrange("p (k f) -> p k f", f=P),
                            in0=io_b, in1=dr_b, op=Alu.is_equal)
                        for j in range(CHUNK):
                            t = tb + j
                            tq = t % tqb            # tile index within cell
                            cell = t // tqb
                            blk = cell % NBLK
                            if tq == 0:
                                cur = ppagg.tile([IN_DIM, P], f32, space="PSUM",
                                                 tag="agg")
                                _aggps[blk] = cur
                            cur = _aggps[blk]
                            last = (tq == tqb - 1)
                            nc.tensor.matmul(
                                out=cur[:],
                                lhsT=msg[:, j * IN_DIM:(j + 1) * IN_DIM],
                                rhs=s_t[:, j * P:(j + 1) * P],
                                start=(tq == 0),
                                stop=last and qi != Q - 1)
                            if last and qi == Q - 1:
                                # fold in (1+eps)*x for this block
                                xb = wpool.tile([P, IN_DIM], f32, tag="xb")
                                nc.sync.dma_start(
                                    out=xb[:],
                                    in_=xloc[blk * P:(blk + 1) * P, :])
                                nc.tensor.matmul(out=cur[:], lhsT=xb[:],
                                                 rhs=ieps_t[:],
                                                 start=False, stop=True)
                            if last:
                                eng = nc.vector
                                asl = agg_sb[:, blk * P:(blk + 1) * P]
                                eng.tensor_tensor(out=asl, in0=asl, in1=cur[:],
                                                  op=Alu.add)

            # ---- MLP tail (feature-major) ----
            MT = 512
            nmt = (NLOC + MT - 1) // MT
            h1_sb = bigpool.tile([IN_DIM, NLOC], f32)
            sum_parts = cpool.tile([IN_DIM, nmt], f32)
            sq_parts = cpool.tile([IN_DIM, nmt], f32)
            sq_scratch = wpool.tile([IN_DIM, MT], f32, tag="sqs")
            for m in range(nmt):
                lo = m * MT
                w = min(MT, NLOC - lo)
                ps1 = pp.tile([IN_DIM, MT], f32, space="PSUM", tag="mlp")
                nc.tensor.matmul(out=ps1[:, :w], lhsT=w1_t[:],
                                 rhs=agg_sb[:, lo:lo + w], start=True, stop=True)
                nc.scalar.activation(out=h1_sb[:, lo:lo + w], in_=ps1[:, :w],
                                     func=Act.Identity, bias=0.0, scale=1.0,
                                     accum_out=sum_parts[:, m:m + 1])
                nc.scalar.activation(out=sq_scratch[:, :w], in_=h1_sb[:, lo:lo + w],
                                     func=Act.Square,
                                     accum_out=sq_parts[:, m:m + 1])
            sums = cpool.tile([IN_DIM, 1], f32)
            nc.vector.tensor_reduce(out=sums[:], in_=sum_parts[:],
                                    axis=mybir.AxisListType.X, op=Alu.add)
            sqs = cpool.tile([IN_DIM, 1], f32)
            nc.vector.tensor_reduce(out=sqs[:], in_=sq_parts[:],
                                    axis=mybir.AxisListType.X, op=Alu.add)
            stats = cpool.tile([IN_DIM, 2], f32)
            nc.vector.tensor_copy(out=stats[:, 0:1], in_=sums[:])
            nc.vector.tensor_copy(out=stats[:, 1:2], in_=sqs[:])
            nc.gpsimd.dma_start(out=cc_in[:], in_=stats[:])
            nc.gpsimd.collective_compute(
                "AllReduce", Alu.add,
                replica_groups=[list(range(NCORES))],
                ins=[cc_in[:]], outs=[cc_out[:]])
            astats = cpool.tile([IN_DIM, 2], f32)
            nc.gpsimd.dma_start(out=astats[:], in_=cc_out[:])

            inv_n = 1.0 / float(N_NODES)
            mu = cpool.tile([IN_DIM, 1], f32)
            nc.vector.tensor_scalar(mu[:], astats[:, 0:1], inv_n, None, Alu.mult)
            ex2 = cpool.tile([IN_DIM, 1], f32)
            nc.vector.tensor_scalar(ex2[:], astats[:, 1:2], inv_n, None, Alu.mult)
            mu2 = cpool.tile([IN_DIM, 1], f32)
            nc.vector.tensor_tensor(out=mu2[:], in0=mu[:], in1=mu[:], op=Alu.mult)
            vare = cpool.tile([IN_DIM, 1], f32)
            nc.vector.tensor_tensor(out=vare[:], in0=ex2[:], in1=mu2[:],
                                    op=Alu.subtract)
            nc.vector.tensor_scalar(vare[:], vare[:], BN_EPS, None, Alu.add)
            rvar = cpool.tile([IN_DIM, 1], f32)
            nc.vector.reciprocal(out=rvar[:], in_=vare[:])
            rstd = cpool.tile([IN_DIM, 1], f32)
            nc.scalar.activation(out=rstd[:], in_=rvar[:], func=Act.Sqrt)
            rg = cpool.tile([IN_DIM, 1], f32)
            nc.vector.tensor_tensor(out=rg[:], in0=rstd[:], in1=gam_t[:],
                                    op=Alu.mult)
            murg = cpool.tile([IN_DIM, 1], f32)
            nc.vector.tensor_tensor(out=murg[:], in0=mu[:], in1=rg[:],
                                    op=Alu.mult)
            bmr = cpool.tile([IN_DIM, 1], f32)
            nc.vector.tensor_tensor(out=bmr[:], in0=bet_t[:], in1=murg[:],
                                    op=Alu.subtract)

            for m in range(nmt):
                lo = m * MT
                w = min(MT, NLOC - lo)
                hr = wpool.tile([IN_DIM, MT], f32, tag="hr")
                nc.scalar.activation(out=hr[:, :w], in_=h1_sb[:, lo:lo + w],
                                     func=Act.Relu, bias=bmr[:, :1],
                                     scale=rg[:, :1])
                ps2 = pp.tile([IN_DIM, MT], f32, space="PSUM", tag="mlp")
                nc.tensor.matmul(out=ps2[:, :w], lhsT=w2_t[:], rhs=hr[:, :w],
                                 start=True, stop=True)
                nc.scalar.activation(out=agg_sb[:, lo:lo + w], in_=ps2[:, :w],
                                     func=Act.Identity, bias=b2_t[:, :1],
                                     scale=1.0)
            nc.sync.dma_start(out=out_d[:], in_=agg_sb[:])

    nc.compile()
    return nc


def _install_ntff_hook():
    """Best-effort NTFF profiling hook (axon terminal). Trace-mode only."""
    import types
    try:
        import antenv
        if not hasattr(antenv, "axon_hooks"):
            m = types.ModuleType("antenv.axon_hooks")
            m._hook = None
            m.set_axon_ntff_profile_hook = lambda h: setattr(m, "_hook", h)
            m.get_axon_ntff_profile_hook = lambda: m._hook
            sys.modules["antenv.axon_hooks"] = m
            antenv.axon_hooks = m
        from antenv import axon_hooks
        if axon_hooks.get_axon_ntff_profile_hook() is None:
            from trn_agent_boot.trn_boot import _ntff_profile_via_ctypes
            h = _ntff_profile_via_ctypes("/opt/axon/libaxon_pjrt.so")
            if h is not None:
                axon_hooks.set_axon_ntff_profile_hook(h)
    except Exception as e:
        print("ntff hook install failed:", e)


def kernel(**inputs):
    x = np.ascontiguousarray(np.asarray(inputs["x"], dtype=np.float32))
    edge_index = np.asarray(inputs["edge_index"])
    edge_attr = np.asarray(inputs["edge_attr"])
    emb = np.ascontiguousarray(np.asarray(inputs["edge_emb_table"], np.float32))
    eps = float(np.asarray(inputs["eps"], np.float32))
    W1 = np.ascontiguousarray(np.asarray(inputs["W1"], np.float32))
    b1 = np.asarray(inputs["b1"], np.float32)  # cancels in BatchNorm; unused
    gamma = np.asarray(inputs["gamma"], np.float32)
    beta = np.asarray(inputs["beta"], np.float32)
    W2 = np.ascontiguousarray(np.asarray(inputs["W2"], np.float32))
    b2 = np.asarray(inputs["b2"], np.float32)

    idx_dram, dstrel_mat, attr1h, meta = _host_prepare(x, edge_index, edge_attr)

    key = (meta["tqb"], meta["T"], meta["calls_q"])
    if key not in _PROGRAM_CACHE:
        _PROGRAM_CACHE[key] = _build_program(meta)
    nc = _PROGRAM_CACHE[key]

    xg = np.zeros((XROWS, IN_DIM), np.float32)
    xg[:N_NODES] = x
    iota = np.tile(np.arange(P, dtype=np.float32), (P, 1)).astype(BF16)
    ident = np.eye(P, dtype=np.float32)
    eps_col = np.full((P, 1), eps, np.float32)

    in_maps = []
    for c in range(NCORES):
        xloc = np.zeros((NLOC, IN_DIM), np.float32)
        xloc[:NPC] = x[c * NPC:(c + 1) * NPC]
        in_maps.append({
            "xg": xg,
            "xloc": xloc,
            "idx16": idx_dram[c],
            "dstrel": dstrel_mat[c],
            "attr1h": attr1h[c],
            "iota": iota,
            "ident": ident,
            "emb": emb,
            "w1": W1,
            "w2": W2,
            "gam": np.ascontiguousarray(gamma.reshape(IN_DIM, 1)),
            "bet": np.ascontiguousarray(beta.reshape(IN_DIM, 1)),
            "b2": np.ascontiguousarray(b2.reshape(IN_DIM, 1)),
            "eps": eps_col,
        })

    from concourse.bass_utils import run_bass_kernel_spmd
    trace = os.environ.get("BASS_GNN_TRACE", "0") == "1"
    if trace:
        _install_ntff_hook()

    res = run_bass_kernel_spmd(nc, in_maps, core_ids=list(range(NCORES)),
                               trace=trace)
    kernel.last_exec_time_ns = res.exec_time_ns
    kernel.last_results = res

    out = np.empty((N_NODES, IN_DIM), np.float32)
    for c in range(NCORES):
        out[c * NPC:(c + 1) * NPC] = res.results[c]["outT"][:, :NPC].T
    return out



# revision 9
# speedup vs baseline: 2.9549x; 2.9549x over previous
"""Trainium2 Bass kernel for CustomGINE message passing (8 NeuronCores), v4.

Like v2 but PSUM-correct: one aggregation chain per PSUM bank.  Dst blocks
are processed in groups of 4 (25 groups, blocks padded 98->100); each
block's chain gets its own [64, 128] PSUM bank.  Edge layout is static
{5,4,5,4} tiles per (block, supergroup) with host-side rebalancing of the
supergroup-overlap edges.  Gathers run on 2 SWDGE queues.  The S scatter
matrix is built with a materialized iota constant (fast DVE path).  The
edge-attr embedding is added chunk-wise with one block-diagonal matmul
pair (bf16 hi/lo).  W1 + BatchNorm stats run per group straight out of
PSUM; the BN apply + W2 pass follows a [64, 2] AllReduce.
"""

import os
import sys

for _p in ("/opt/trn_rl_repo", "/root/.axon_site/_ro/trn_rl_repo"):
    if os.path.isdir(_p) and _p not in sys.path:
        sys.path.insert(0, _p)

import numpy as np
import ml_dtypes

BF16 = ml_dtypes.bfloat16

N_NODES = 100000
IN_DIM = 64
NCORES = 8
NPC = N_NODES // NCORES          # 12500
P = 128
NBLK = (NPC + P - 1) // P        # 98 real dst blocks
NBLKP = 100                      # padded to a multiple of GB
GB = 4                           # dst blocks per group (one PSUM bank each)
NWG = NBLKP // GB                # 25 window groups
NLOC = NBLKP * P                 # 12800
Q = 4
QS = 25000
QW = 32768
XROWS = 3 * QS + QW
BN_EPS = 1e-5

_PROGRAM_CACHE = {}


def _host_prepare(x, edge_index, edge_attr):
    src = np.asarray(edge_index[0], dtype=np.int64)
    dst = np.asarray(edge_index[1], dtype=np.int64)
    attr = np.asarray(edge_attr, dtype=np.int64)
    E = src.shape[0]

    core = dst // NPC
    dloc = dst - core * NPC
    b = dloc >> 7
    cell = core * NBLK + b

    zone = np.zeros(E, dtype=np.int64)
    zone[src >= 25000] = 1
    zone[src >= 32768] = 2
    zone[src >= 50000] = 3
    zone[src >= 57768] = 4
    zone[src >= 75000] = 5
    zone[src >= 82768] = 6

    ncell = NCORES * NBLK
    zc = np.zeros((ncell, 7), dtype=np.int64)
    np.add.at(zc, (cell, zone), 1)

    tpat = (5, 4, 5, 4)
    for _try in range(3):
        caps = np.array([t * P for t in tpat], dtype=np.int64)
        x01 = np.clip(caps[0] - zc[:, 0], 0, zc[:, 1])
        n1_forced = zc[:, 2] + (zc[:, 1] - x01)
        x12 = np.clip(caps[1] - n1_forced, 0, zc[:, 3])
        n2_forced = zc[:, 4] + (zc[:, 3] - x12)
        x23 = np.clip(caps[2] - n2_forced, 0, zc[:, 5])
        n3 = zc[:, 6] + (zc[:, 5] - x23)
        ok = ((zc[:, 0] <= caps[0]) & (n1_forced <= caps[1] + x12 - x12)
              & (n2_forced <= caps[2] + x23 - x23) & (n3 <= caps[3]))
        if ok.all():
            break
        worst = int(np.argmax([(zc[:, 0]).max() / caps[0],
                               n1_forced.max() / caps[1],
                               n2_forced.max() / caps[2],
                               n3.max() / caps[3]]))
        tpat = tuple(t + (1 if i == worst else 0) for i, t in enumerate(tpat))
    else:
        raise RuntimeError("could not balance supergroup cells")

    q = np.array([0, 0, 1, 1, 2, 2, 3], dtype=np.int64)[zone]
    okey = cell * 7 + zone
    order0 = np.argsort(okey, kind="stable")
    ro = np.empty(E, dtype=np.int64)
    ro[order0] = np.arange(E)
    start = np.zeros(ncell * 7 + 1, dtype=np.int64)
    np.add.at(start, okey + 1, 1)
    start = np.cumsum(start)
    rank = ro - start[okey]
    sel = zone == 1
    q[sel] = np.where(rank[sel] < x01[cell[sel]], 0, 1)
    sel = zone == 3
    q[sel] = np.where(rank[sel] < x12[cell[sel]], 1, 2)
    sel = zone == 5
    q[sel] = np.where(rank[sel] < x23[cell[sel]], 2, 3)

    qc = np.zeros((ncell, 4), dtype=np.int64)
    np.add.at(qc, (cell, q), 1)
    caps = np.array([t * P for t in tpat], dtype=np.int64)
    assert (qc <= caps[None, :]).all(), "cell capacity exceeded"

    TPB = sum(tpat)                   # 18 tiles per block
    SLOTS = NBLKP * TPB * P           # per core, includes 2 pad blocks
    T = NBLKP * TPB
    capq = caps
    qoff = np.concatenate([[0], np.cumsum(capq * GB)])
    wg_sz = int(qoff[-1])             # slots per window group (9216)
    wg = b // GB
    bg = b % GB

    slot_base = wg * wg_sz + qoff[q] + bg * capq[q]
    okey2 = cell * 4 + q
    order1 = np.argsort(okey2, kind="stable")
    ro1 = np.empty(E, dtype=np.int64)
    ro1[order1] = np.arange(E)
    start2 = np.zeros(ncell * 4 + 1, dtype=np.int64)
    np.add.at(start2, okey2 + 1, 1)
    start2 = np.cumsum(start2)
    pos = ro1 - start2[okey2]
    slot = slot_base + pos

    # chunk structure per (wg, q): tiles = tpat[q]*GB, chunk sizes <= 8
    def chunks_for(tq):
        n = tq * GB
        out = []
        while n > 8:
            out.append(8)
            n -= 8
        out.append(n)
        return out

    chk = [chunks_for(t) for t in tpat]          # per q
    nch_q = [len(c) for c in chk]
    NCHG = sum(nch_q)                            # chunks per wg
    qtileoff = np.concatenate([[0], np.cumsum([t * GB for t in tpat])])
    qchoff = np.concatenate([[0], np.cumsum(nch_q)])
    # per (q, t_local) -> (chunk_local, k_in_chunk)
    t2ck = []
    for qi2 in range(4):
        m = []
        cb = 0
        for ci, csz in enumerate(chk[qi2]):
            for k in range(csz):
                m.append((ci, k))
            cb += csz
        t2ck.append(np.array(m, dtype=np.int64))

    t_local = (slot - (wg * wg_sz + qoff[q])) // P
    lane = slot % P
    ck = np.zeros((E, 2), dtype=np.int64)
    for qi2 in range(4):
        selq = q == qi2
        ck[selq] = t2ck[qi2][t_local[selq]]
    chunk_g = wg * NCHG + qchoff[q] + ck[:, 0]
    NCH = NWG * NCHG                              # chunks per core
    MAXC = 8

    idx16 = np.zeros((NCORES, SLOTS), dtype=np.int16)
    dstrel = np.full((NCORES, SLOTS), -1.0, dtype=BF16)
    attr32 = np.zeros((NCORES, 4 * MAXC, NCH * P), dtype=BF16)

    idx16[core, slot] = (src - q * QS).astype(np.int16)
    dstrel[core, slot] = (dloc & 127).astype(np.float32).astype(BF16)
    attr32[core, 4 * ck[:, 1] + attr, chunk_g * P + lane] = BF16(1.0)

    # idx wrapped per gather call (wg, q)
    idx_dram = np.zeros((NCORES, P, SLOTS // 16), dtype=np.int16)
    colpos = 0
    call_info = []
    for wgi in range(NWG):
        for qi2 in range(4):
            cs = int(capq[qi2]) * GB
            s0 = wgi * wg_sz + int(qoff[qi2])
            blk = idx16[:, s0:s0 + cs].reshape(NCORES, cs // 16, 16)
            blk = np.swapaxes(blk, 1, 2)
            idx_dram[:, :, colpos:colpos + cs // 16] = np.tile(blk, (1, 8, 1))
            call_info.append((s0, cs, colpos))
            colpos += cs // 16
    assert colpos == SLOTS // 16

    dstrel_mat = np.ascontiguousarray(
        dstrel.reshape(NCORES, T, P).swapaxes(1, 2))

    meta = dict(tpat=tpat, T=T, SLOTS=SLOTS, call_info=call_info,
                wg_sz=wg_sz, capq=tuple(int(c) for c in capq),
                chk=chk, NCHG=NCHG, NCH=NCH,
                qtileoff=[int(v) for v in qtileoff],
                qchoff=[int(v) for v in qchoff])
    return idx_dram, dstrel_mat, attr32, meta


def _build_program(meta):
    import concourse.bass as bass
    import concourse.bacc as bacc
    import concourse.mybir as mybir
    import concourse.tile as tile

    dt = mybir.dt
    Alu = mybir.AluOpType
    Act = mybir.ActivationFunctionType

    tpat = meta["tpat"]
    T = meta["T"]
    SLOTS = meta["SLOTS"]
    call_info = meta["call_info"]
    capq = meta["capq"]
    chk = meta["chk"]
    NCH = meta["NCH"]
    qtileoff = meta["qtileoff"]
    TPB = sum(tpat)
    MAXC = 8

    nc = bacc.Bacc("TRN2", target_bir_lowering=False, debug=False,
                   num_devices=NCORES, num_swdge_queues=2)

    f32, bf16, i16 = dt.float32, dt.bfloat16, dt.int16

    xg = nc.dram_tensor("xg", [XROWS, IN_DIM], f32, kind="ExternalInput")
    xloct = nc.dram_tensor("xloct", [P, NBLKP * IN_DIM], f32,
                           kind="ExternalInput")
    idx16 = nc.dram_tensor("idx16", [P, SLOTS // 16], i16, kind="ExternalInput")
    dstrel = nc.dram_tensor("dstrel", [P, T], bf16, kind="ExternalInput")
    attr32 = nc.dram_tensor("attr32", [4 * MAXC, NCH * P], bf16,
                            kind="ExternalInput")
    iota8_d = nc.dram_tensor("iota8", [P, MAXC * P], bf16,
                             kind="ExternalInput")
    ident_d = nc.dram_tensor("ident", [P, P], f32, kind="ExternalInput")
    embbd_d = nc.dram_tensor("embbd", [4 * MAXC, MAXC * IN_DIM], f32,
                             kind="ExternalInput")
    w1_d = nc.dram_tensor("w1", [IN_DIM, IN_DIM], f32, kind="ExternalInput")
    w2_d = nc.dram_tensor("w2", [IN_DIM, IN_DIM], f32, kind="ExternalInput")
    gam_d = nc.dram_tensor("gam", [IN_DIM, 1], f32, kind="ExternalInput")
    bet_d = nc.dram_tensor("bet", [IN_DIM, 1], f32, kind="ExternalInput")
    b2_d = nc.dram_tensor("b2", [IN_DIM, 1], f32, kind="ExternalInput")
    eps_d = nc.dram_tensor("eps", [P, 1], f32, kind="ExternalInput")
    out_d = nc.dram_tensor("outT", [IN_DIM, NLOC], f32, kind="ExternalOutput")

    cc_in = nc.dram_tensor("cc_in", [IN_DIM, 2], f32)
    cc_out = nc.dram_tensor("cc_out", [IN_DIM, 2], f32, addr_space="Shared")

    MT = GB * P                       # 512
    NMT = NWG                         # 25

    with tile.TileContext(nc) as tc:
        with (
            tc.tile_pool(name="const", bufs=1) as cpool,
            tc.tile_pool(name="big", bufs=1) as bigpool,
            tc.tile_pool(name="gin", bufs=3) as gpool,
            tc.tile_pool(name="meta", bufs=2) as mpool,
            tc.tile_pool(name="work", bufs=3) as wpool,
            tc.tile_pool(name="aggs", bufs=2) as apool,
            tc.tile_pool(name="pp", bufs=2, space="PSUM") as pp,
            tc.tile_pool(name="ps1", bufs=2, space="PSUM") as pp1,
            tc.tile_pool(name="psagg", bufs=4, space="PSUM") as ppagg,
        ):
            iota8_t = cpool.tile([P, MAXC * P], bf16)
            nc.sync.dma_start(out=iota8_t[:], in_=iota8_d[:])
            ident_t = cpool.tile([P, P], f32)
            nc.sync.dma_start(out=ident_t[:], in_=ident_d[:])
            dstrel_t = cpool.tile([P, T], bf16)
            nc.sync.dma_start(out=dstrel_t[:], in_=dstrel[:])
            embbd_t = cpool.tile([4 * MAXC, MAXC * IN_DIM], f32)
            nc.sync.dma_start(out=embbd_t[:], in_=embbd_d[:])
            w1_t = cpool.tile([IN_DIM, IN_DIM], f32)
            nc.sync.dma_start(out=w1_t[:], in_=w1_d[:])
            w2_t = cpool.tile([IN_DIM, IN_DIM], f32)
            nc.sync.dma_start(out=w2_t[:], in_=w2_d[:])
            gam_t = cpool.tile([IN_DIM, 1], f32)
            nc.sync.dma_start(out=gam_t[:], in_=gam_d[:])
            bet_t = cpool.tile([IN_DIM, 1], f32)
            nc.sync.dma_start(out=bet_t[:], in_=bet_d[:])
            b2_t = cpool.tile([IN_DIM, 1], f32)
            nc.sync.dma_start(out=b2_t[:], in_=b2_d[:])
            eps_t = cpool.tile([P, 1], f32)
            nc.sync.dma_start(out=eps_t[:], in_=eps_d[:])
            xloc_t = cpool.tile([P, NBLKP * IN_DIM], f32)
            nc.sync.dma_start(out=xloc_t[:], in_=xloct[:])

            ebd_hi = cpool.tile([4 * MAXC, MAXC * IN_DIM], bf16)
            nc.vector.tensor_copy(out=ebd_hi[:], in_=embbd_t[:])
            ebd_hi_f = cpool.tile([4 * MAXC, MAXC * IN_DIM], f32)
            nc.vector.tensor_copy(out=ebd_hi_f[:], in_=ebd_hi[:])
            ebd_lo_f = cpool.tile([4 * MAXC, MAXC * IN_DIM], f32)
            nc.vector.tensor_tensor(out=ebd_lo_f[:], in0=embbd_t[:],
                                    in1=ebd_hi_f[:], op=Alu.subtract)
            ebd_lo = cpool.tile([4 * MAXC, MAXC * IN_DIM], bf16)
            nc.vector.tensor_copy(out=ebd_lo[:], in_=ebd_lo_f[:])

            eps1_t = cpool.tile([P, 1], f32)
            nc.scalar.activation(out=eps1_t[:], in_=eps_t[:], func=Act.Identity,
                                 bias=1.0, scale=1.0)
            ieps_t = cpool.tile([P, P], f32)
            nc.vector.tensor_scalar(ieps_t[:], ident_t[:], eps1_t[:, :1], None,
                                    Alu.mult)

            h1_sb = bigpool.tile([IN_DIM, NLOC], f32)
            sum_parts = cpool.tile([IN_DIM, NMT], f32)
            sq_parts = cpool.tile([IN_DIM, NMT], f32)
            sq_scratch = wpool.tile([IN_DIM, MT], f32, tag="sqs")

            gctr = 0
            for wgi in range(NWG):
                aggps = [ppagg.tile([IN_DIM, P], f32, space="PSUM", tag="agg",
                                    name=f"aggps_w{wgi}b{i}")
                         for i in range(GB)]
                for qi in range(Q):
                    call = wgi * 4 + qi
                    s0, cs, colpos = call_info[call]
                    idx_sb = mpool.tile([P, cs // 16], i16, tag="idx")
                    nc.sync.dma_start(
                        out=idx_sb[:],
                        in_=idx16[:, colpos:colpos + cs // 16])
                    nchq = len(chk[qi])
                    ch0 = (wgi * meta["NCHG"] + meta["qchoff"][qi])
                    at_sb = mpool.tile([4 * MAXC, nchq * P], bf16, tag="attr")
                    nc.sync.dma_start(
                        out=at_sb[:],
                        in_=attr32[:, ch0 * P:(ch0 + nchq) * P])
                    gbuf = gpool.tile([P, (cs // P) * IN_DIM], f32, tag="g")
                    nc.gpsimd.dma_gather(
                        out_ap=gbuf[:].rearrange("p (k f) -> p k f", f=IN_DIM),
                        in_ap=xg[qi * QS:qi * QS + QW, :],
                        idxs_ap=idx_sb[:],
                        num_idxs=cs,
                        num_idxs_reg=cs,
                        elem_size=IN_DIM,
                        single_packet=False,
                        queue_num=gctr % 2,
                    )
                    gctr += 1
                    tq0 = wgi * TPB * GB + qtileoff[qi]  # first global tile
                    cb = 0
                    for ci, csz in enumerate(chk[qi]):
                        tb = tq0 + cb          # global tile of chunk start
                        ps_pre = pp.tile([P, MAXC * IN_DIM], f32, space="PSUM",
                                         tag="pre")
                        lhs = at_sb[:4 * csz, ci * P:(ci + 1) * P]
                        nc.tensor.matmul(out=ps_pre[:, :csz * IN_DIM],
                                         lhsT=lhs,
                                         rhs=ebd_hi[:4 * csz, :csz * IN_DIM],
                                         start=True, stop=False)
                        nc.tensor.matmul(out=ps_pre[:, :csz * IN_DIM],
                                         lhsT=lhs,
                                         rhs=ebd_lo[:4 * csz, :csz * IN_DIM],
                                         start=False, stop=True)
                        gsl = gbuf[:, cb * IN_DIM:(cb + csz) * IN_DIM]
                        madd = wpool.tile([P, MAXC * IN_DIM], f32, tag="madd")
                        nc.vector.tensor_tensor(out=madd[:, :csz * IN_DIM],
                                                in0=gsl,
                                                in1=ps_pre[:, :csz * IN_DIM],
                                                op=Alu.add)
                        msg = wpool.tile([P, MAXC * IN_DIM], bf16, tag="msg")
                        nc.scalar.activation(out=msg[:, :csz * IN_DIM],
                                             in_=madd[:, :csz * IN_DIM],
                                             func=Act.Relu)
                        s_t = wpool.tile([P, MAXC * P], bf16, tag="S")
                        dr_b = dstrel_t[:, tb:tb + csz].rearrange(
                            "p (k o) -> p k o", o=1).to_broadcast([P, csz, P])
                        nc.vector.tensor_tensor(
                            out=s_t[:, :csz * P].rearrange(
                                "p (k f) -> p k f", f=P),
                            in0=iota8_t[:, :csz * P].rearrange(
                                "p (k f) -> p k f", f=P),
                            in1=dr_b, op=Alu.is_equal)
                        for j in range(csz):
                            tl = cb + j           # t_local within (wg, q)
                            bgj = tl // tpat[qi]
                            tqj = tl % tpat[qi]
                            first = (qi == 0 and tqj == 0)
                            nc.tensor.matmul(
                                out=aggps[bgj][:],
                                lhsT=msg[:, j * IN_DIM:(j + 1) * IN_DIM],
                                rhs=s_t[:, j * P:(j + 1) * P],
                                start=first, stop=False)
                        cb += csz
                # fold (1+eps)*x, close chains
                for bg in range(GB):
                    blk = wgi * GB + bg
                    nc.tensor.matmul(
                        out=aggps[bg][:],
                        lhsT=xloc_t[:, blk * IN_DIM:(blk + 1) * IN_DIM],
                        rhs=ieps_t[:],
                        start=False, stop=True)
                aggsb = apool.tile([IN_DIM, MT], f32, tag="aggsb")
                for bg in range(GB):
                    nc.scalar.activation(out=aggsb[:, bg * P:(bg + 1) * P],
                                         in_=aggps[bg][:], func=Act.Identity,
                                         bias=0.0, scale=1.0)
                ps1 = pp1.tile([IN_DIM, MT], f32, space="PSUM", tag="mlp")
                nc.tensor.matmul(out=ps1[:], lhsT=w1_t[:], rhs=aggsb[:],
                                 start=True, stop=True)
                glo = wgi * MT
                nc.scalar.activation(out=h1_sb[:, glo:glo + MT],
                                     in_=ps1[:], func=Act.Identity,
                                     bias=0.0, scale=1.0,
                                     accum_out=sum_parts[:, wgi:wgi + 1])
                nc.scalar.activation(out=sq_scratch[:],
                                     in_=h1_sb[:, glo:glo + MT],
                                     func=Act.Square,
                                     accum_out=sq_parts[:, wgi:wgi + 1])

            sums = cpool.tile([IN_DIM, 1], f32)
            nc.vector.tensor_reduce(out=sums[:], in_=sum_parts[:],
                                    axis=mybir.AxisListType.X, op=Alu.add)
            sqs = cpool.tile([IN_DIM, 1], f32)
            nc.vector.tensor_reduce(out=sqs[:], in_=sq_parts[:],
                                    axis=mybir.AxisListType.X, op=Alu.add)
            stats = cpool.tile([IN_DIM, 2], f32)
            nc.vector.tensor_copy(out=stats[:, 0:1], in_=sums[:])
            nc.vector.tensor_copy(out=stats[:, 1:2], in_=sqs[:])
            nc.gpsimd.dma_start(out=cc_in[:], in_=stats[:])
            nc.gpsimd.collective_compute(
                "AllReduce", Alu.add,
                replica_groups=[list(range(NCORES))],
                ins=[cc_in[:]], outs=[cc_out[:]])
            astats = cpool.tile([IN_DIM, 2], f32)
            nc.gpsimd.dma_start(out=astats[:], in_=cc_out[:])

            inv_n = 1.0 / float(N_NODES)
            mu = cpool.tile([IN_DIM, 1], f32)
            nc.vector.tensor_scalar(mu[:], astats[:, 0:1], inv_n, None,
                                    Alu.mult)
            ex2 = cpool.tile([IN_DIM, 1], f32)
            nc.vector.tensor_scalar(ex2[:], astats[:, 1:2], inv_n, None,
                                    Alu.mult)
            mu2 = cpool.tile([IN_DIM, 1], f32)
            nc.vector.tensor_tensor(out=mu2[:], in0=mu[:], in1=mu[:],
                                    op=Alu.mult)
            vare = cpool.tile([IN_DIM, 1], f32)
            nc.vector.tensor_tensor(out=vare[:], in0=ex2[:], in1=mu2[:],
                                    op=Alu.subtract)
            nc.vector.tensor_scalar(vare[:], vare[:], BN_EPS, None, Alu.add)
            rvar = cpool.tile([IN_DIM, 1], f32)
            nc.vector.reciprocal(out=rvar[:], in_=vare[:])
            rstd = cpool.tile([IN_DIM, 1], f32)
            nc.scalar.activation(out=rstd[:], in_=rvar[:], func=Act.Sqrt)
            rg = cpool.tile([IN_DIM, 1], f32)
            nc.vector.tensor_tensor(out=rg[:], in0=rstd[:], in1=gam_t[:],
                                    op=Alu.mult)
            murg = cpool.tile([IN_DIM, 1], f32)
            nc.vector.tensor_tensor(out=murg[:], in0=mu[:], in1=rg[:],
                                    op=Alu.mult)
            bmr = cpool.tile([IN_DIM, 1], f32)
            nc.vector.tensor_tensor(out=bmr[:], in0=bet_t[:], in1=murg[:],
                                    op=Alu.subtract)

            for m in range(NMT):
                lo = m * MT
                hr = wpool.tile([IN_DIM, MT], f32, tag="hr")
                nc.scalar.activation(out=hr[:], in_=h1_sb[:, lo:lo + MT],
                                     func=Act.Relu, bias=bmr[:, :1],
                                     scale=rg[:, :1])
                ps2 = pp1.tile([IN_DIM, MT], f32, space="PSUM", tag="mlp")
                nc.tensor.matmul(out=ps2[:], lhsT=w2_t[:], rhs=hr[:],
                                 start=True, stop=True)
                ot = wpool.tile([IN_DIM, MT], f32, tag="ot")
                nc.scalar.activation(out=ot[:], in_=ps2[:],
                                     func=Act.Identity, bias=b2_t[:, :1],
                                     scale=1.0)
                nc.sync.dma_start(out=out_d[:, lo:lo + MT], in_=ot[:])

    nc.compile()
    return nc


def _install_ntff_hook():
    import types
    try:
        import antenv
        if not hasattr(antenv, "axon_hooks"):
            m = types.ModuleType("antenv.axon_hooks")
            m._hook = None
            m.set_axon_ntff_profile_hook = lambda h: setattr(m, "_hook", h)
            m.get_axon_ntff_profile_hook = lambda: m._hook
            sys.modules["antenv.axon_hooks"] = m
            antenv.axon_hooks = m
        from antenv import axon_hooks
        if axon_hooks.get_axon_ntff_profile_hook() is None:
            from trn_agent_boot.trn_boot import _ntff_profile_via_ctypes
            h = _ntff_profile_via_ctypes("/opt/axon/libaxon_pjrt.so")
            if h is not None:
                axon_hooks.set_axon_ntff_profile_hook(h)
    except Exception as e:
        print("ntff hook install failed:", e)


def kernel(**inputs):
    x = np.ascontiguousarray(np.asarray(inputs["x"], dtype=np.float32))
    edge_index = np.asarray(inputs["edge_index"])
    edge_attr = np.asarray(inputs["edge_attr"])
    emb = np.ascontiguousarray(np.asarray(inputs["edge_emb_table"], np.float32))
    eps = float(np.asarray(inputs["eps"], np.float32))
    gamma = np.asarray(inputs["gamma"], np.float32)
    beta = np.asarray(inputs["beta"], np.float32)
    W1 = np.ascontiguousarray(np.asarray(inputs["W1"], np.float32))
    W2 = np.ascontiguousarray(np.asarray(inputs["W2"], np.float32))
    b2 = np.asarray(inputs["b2"], np.float32)
    # b1 cancels in BatchNorm; unused

    idx_dram, dstrel_mat, attr32, meta = _host_prepare(x, edge_index, edge_attr)

    key = (meta["tpat"], meta["T"])
    if key not in _PROGRAM_CACHE:
        _PROGRAM_CACHE[key] = _build_program(meta)
    nc = _PROGRAM_CACHE[key]

    MAXC = 8
    xg = np.zeros((XROWS, IN_DIM), np.float32)
    xg[:N_NODES] = x
    iota = np.tile(np.arange(P, dtype=np.float32), (P, 1)).astype(BF16)
    iota8 = np.tile(iota, (1, MAXC))
    ident = np.eye(P, dtype=np.float32)
    eps_col = np.full((P, 1), eps, np.float32)
    embbd = np.zeros((4 * MAXC, MAXC * IN_DIM), np.float32)
    for k in range(MAXC):
        embbd[4 * k:4 * k + 4, IN_DIM * k:IN_DIM * (k + 1)] = emb

    in_maps = []
    for c in range(NCORES):
        xloc = np.zeros((NLOC, IN_DIM), np.float32)
        xloc[:NPC] = x[c * NPC:(c + 1) * NPC]
        xloct = np.ascontiguousarray(
            xloc.reshape(NBLKP, P, IN_DIM).transpose(1, 0, 2).reshape(
                P, NBLKP * IN_DIM))
        in_maps.append({
            "xg": xg,
            "xloct": xloct,
            "idx16": idx_dram[c],
            "dstrel": dstrel_mat[c],
            "attr32": attr32[c],
            "iota8": iota8,
            "ident": ident,
            "embbd": embbd,
            "w1": W1,
            "w2": W2,
            "gam": np.ascontiguousarray(gamma.reshape(IN_DIM, 1)),
            "bet": np.ascontiguousarray(beta.reshape(IN_DIM, 1)),
            "b2": np.ascontiguousarray(b2.reshape(IN_DIM, 1)),
            "eps": eps_col,
        })

    from concourse.bass_utils import run_bass_kernel_spmd
    trace = os.environ.get("BASS_GNN_TRACE", "0") == "1"
    if trace:
        _install_ntff_hook()

    res = run_bass_kernel_spmd(nc, in_maps, core_ids=list(range(NCORES)),
                               trace=trace)
    kernel.last_exec_time_ns = res.exec_time_ns
    kernel.last_results = res

    out = np.empty((N_NODES, IN_DIM), np.float32)
    for c in range(NCORES):
        out[c * NPC:(c + 1) * NPC] = res.results[c]["outT"][:, :NPC].T
    return out
